# revision 53
# baseline (speedup 1.0000x reference)
"""Multi-head attention layer on 8 trn2 NeuronCores — v4.

Structure: everything except the final out-projection is head-sharded
(2 of 16 heads per core), so the Q/K/V projections write their outputs
directly into the attention layout with NO collectives: each core
projects ALL tokens against its own 128-column weight slice (identical
PE cycles to row-sharding — work/8 either way), reading replicated
full inputs from HBM, which overlaps with compute from t=0.  A single
AllToAll converts the attention output to row-sharding for out_fc; it
carries UNNORMALIZED numerators plus the softmax denominators (row 65
of each head block), and normalization happens after the exchange in
the row-sharded domain (one reciprocal + per-tile broadcast matmuls),
keeping the attention inner loop free of DVE chains.

Key compaction: masked keys contribute exactly 0 to the output
(numerator and denominator), so the host drops them before the K/V
projections, padding per batch to a whole number of 128-key tiles (1024 for the
reference mask, vs S=2048).  Pad keys carry valid=0 which zeroes their
V rows and denominator column, so any mask stays exact; the build is
specialized per key-tile count and cached.

Phase-1 work is interleaved with batch-0 attention: batch-1 K/V
projections and later Q blocks are emitted between attention blocks so
their DMAs hide under compute; attention starts ~20us into the kernel.

Everything is bf16 except PSUM accumulation (fp32) and the fp32
output.  Scores are computed transposed (keys on partitions) so exp is
one ScalarE activation per key tile and attn@V needs no transposes.
"""
import numpy as np

from concourse import bacc, tile, mybir
from concourse.bass_utils import run_bass_kernel_spmd

N_CORES = 8
B, S, D, H = 2, 2048, 1024, 16
DK = D // H                      # 64
R = B * S                        # 4096 token rows
RPC = R // N_CORES               # 512 rows per core (phase-3 row-sharding)
HPC = H // N_CORES               # 2 heads per core
KT = D // 128                    # 8 contraction tiles
NT = D // 128                    # 8 output-dim tiles (out-projection)
QT = S // 512                    # 4 query blocks of 512 per batch

# compacted+padded keys per batch; set from the runtime mask by _prep()
# (ceil(max unmasked per batch / 128) tiles).  The build specializes on it.
SKC = 1024
KTB = SKC // 128                 # key tiles per batch
NKC = B * SKC                    # compacted keys total
KCH = NKC // 8                   # key-chunk width for the K projection


def _set_skc(skc):
    global SKC, KTB, NKC, KCH
    SKC = skc
    KTB = SKC // 128
    NKC = B * SKC
    KCH = NKC // 8

dt = mybir.dt
AF = mybir.ActivationFunctionType

_CACHE = {}


def _build(no_collective=False, reps=1):
    nc = bacc.Bacc("TRN2", target_bir_lowering=False, debug=False,
                   num_devices=N_CORES)

    # ---- kernel I/O (per-core tensors; x* are replicated full inputs,
    # w*/b* slices are this core's 128 head-dims) ----
    xqTf = nc.dram_tensor("xqTf", [D, R], dt.bfloat16, kind="ExternalInput")
    xkTf = nc.dram_tensor("xkTf", [D, NKC], dt.bfloat16, kind="ExternalInput")
    xvTf = nc.dram_tensor("xvTf", [D, NKC], dt.bfloat16, kind="ExternalInput")
    wqs = nc.dram_tensor("wqs", [D, 128], dt.bfloat16, kind="ExternalInput")
    wks = nc.dram_tensor("wks", [D, 128], dt.bfloat16, kind="ExternalInput")
    wvs = nc.dram_tensor("wvs", [D, 128], dt.bfloat16, kind="ExternalInput")
    wo = nc.dram_tensor("wo", [D, D], dt.bfloat16, kind="ExternalInput")
    bqs = nc.dram_tensor("bqs", [128, 1], dt.float32, kind="ExternalInput")
    bks = nc.dram_tensor("bks", [128, 1], dt.float32, kind="ExternalInput")
    bvs = nc.dram_tensor("bvs", [1, 128], dt.bfloat16, kind="ExternalInput")
    bo = nc.dram_tensor("bo", [D], dt.float32, kind="ExternalInput")
    # valid flags of compacted key kt*128+p: bf16 copy feeds v_aug's
    # denominator column, f32 copy feeds the V-row zeroing multiply
    validm = nc.dram_tensor("validm", [128, B * KTB], dt.bfloat16,
                            kind="ExternalInput")
    validf = nc.dram_tensor("validf", [128, B * KTB], dt.float32,
                            kind="ExternalInput")
    # selm[r, t, m] = 1 iff r == t + 8*(m//64): selects this tile's two
    # denominator rows for the phase-3 broadcast matmul
    selm = nc.dram_tensor("selm", [16, KT * 128], dt.float32,
                          kind="ExternalInput")
    onesb = nc.dram_tensor("onesb", [1, 128], dt.bfloat16,
                           kind="ExternalInput")
    outT = nc.dram_tensor("outT", [D, RPC], dt.float32, kind="ExternalOutput")

    f32r = dt.float32r
    bf16 = dt.bfloat16
    rg = [list(range(N_CORES))]

    with tile.TileContext(nc) as tc:
        with tc.tile_pool(name="dram", bufs=1, space="DRAM") as dram:
            for rep in range(reps):
                a2_in = dram.tile([N_CORES, 130, RPC], bf16)
                a2_out = dram.tile([N_CORES, 130, RPC], bf16)

                # out-projection weights (outlive the attention pools;
                # DMAs issued mid-attention so input loads go first)
                pw3 = tc.alloc_tile_pool(name="pw3", bufs=1)
                wo_t = pw3.tile([128, KT, D], bf16, tag="wo")
                bo_sb = pw3.tile([128, NT], dt.float32, tag="bo")
                selm_sb = pw3.tile([16, KT, 128], f32r, tag="selm")
                nc.scalar.dma_start(
                    selm_sb[:],
                    selm[:].rearrange("e (t m) -> e t m",
                                      t=KT).bitcast(f32r))

                with (
                    tc.tile_pool(name="pw", bufs=1) as pw,
                    tc.tile_pool(name="pxk", bufs=3) as pxk,
                    tc.tile_pool(name="pxv", bufs=3) as pxv,
                    tc.tile_pool(name="pxq", bufs=3) as pxq,
                    tc.tile_pool(name="pkv", bufs=1) as pkv,
                    tc.tile_pool(name="pp", bufs=4) as pp,
                    tc.tile_pool(name="psS", bufs=2, space="PSUM") as psS,
                    tc.tile_pool(name="psO", bufs=3, space="PSUM") as psO,
                    tc.tile_pool(name="psP", bufs=1, space="PSUM") as psP,
                ):
                    # ---- weights / biases / flags ----
                    wks_t = pw.tile([128, KT, 128], bf16, tag="wks")
                    wvs_t = pw.tile([128, KT, 128], bf16, tag="wvs")
                    wqs_t = pw.tile([128, KT, 128], bf16, tag="wqs")
                    nc.scalar.dma_start(
                        wks_t[:], wks[:].rearrange("(t p) n -> p t n", p=128))
                    nc.scalar.dma_start(
                        wvs_t[:], wvs[:].rearrange("(t p) n -> p t n", p=128))
                    nc.scalar.dma_start(
                        wqs_t[:], wqs[:].rearrange("(t p) n -> p t n", p=128))
                    bks_sb = pw.tile([128, 1], dt.float32, tag="bks")
                    bqs_sb = pw.tile([128, 1], dt.float32, tag="bqs")
                    bvs_sb = pw.tile([1, 128], bf16, tag="bvs")
                    nc.scalar.dma_start(bks_sb[:], bks[:])
                    nc.scalar.dma_start(bqs_sb[:], bqs[:])
                    nc.scalar.dma_start(bvs_sb[:], bvs[:])
                    onesb_sb = pw.tile([1, 128], bf16, tag="onesb")
                    nc.scalar.dma_start(onesb_sb[:], onesb[:])
                    v01_sb = pw.tile([128, B * KTB], dt.float32, tag="v01")
                    nc.scalar.dma_start(v01_sb[:], validf[:])

                    # ---- attention-layout outputs of the projections ----
                    kT_h = pkv.tile([128, NKC], bf16, tag="kh")
                    qT_h = pkv.tile([128, R], bf16, tag="qh")
                    # v_aug per key tile: [128, 130]: cols h*65+0..63 = V head
                    # h, col h*65+64 = valid flag (softmax denominator)
                    v_aug = pkv.tile([128, B * KTB, 130], bf16, tag="vh")
                    o65 = [pkv.tile([65, R], bf16, tag=f"o65_{h}",
                                    name=f"o65_{h}")
                           for h in range(HPC)]

                    def emit_kchunk(ch):
                        xk_c = pxk.tile([128, KT, KCH], bf16, tag="xk")
                        nc.sync.dma_start(
                            xk_c[:],
                            xkTf[:, ch * KCH:(ch + 1) * KCH]
                            .rearrange("(t p) r -> p t r", p=128))
                        psf = psP.tile([128, 512], dt.float32, tag="p")
                        ps = psf[:, 0:KCH]
                        for t in range(KT):
                            nc.tensor.matmul(
                                ps, wks_t[:, t], xk_c[:, t],
                                start=(t == 0), stop=(t == KT - 1))
                        nc.vector.tensor_scalar_add(
                            kT_h[:, ch * KCH:(ch + 1) * KCH], ps,
                            bks_sb[:, 0:1])

                    def emit_vchunk(kt0, nkt=3):
                        # one batched DMA, then per-tile projections
                        nkt = min(nkt, B * KTB - kt0)
                        xv_c = pxv.tile([128, KT, 3 * 128], bf16, tag="xv")
                        nc.gpsimd.dma_start(
                            xv_c[:, :, 0:nkt * 128],
                            xvTf[:, kt0 * 128:(kt0 + nkt) * 128]
                            .rearrange("(t p) r -> p t r", p=128))
                        for kk in range(nkt):
                            kt = kt0 + kk
                            psf = psP.tile([128, 512], dt.float32, tag="p")
                            ps = psf[:, 0:128]
                            for t in range(KT):
                                nc.tensor.matmul(
                                    ps, xv_c[:, t, kk * 128:(kk + 1) * 128],
                                    wvs_t[:, t],
                                    start=(t == 0), stop=False)
                            nc.tensor.matmul(ps, onesb_sb[:], bvs_sb[:],
                                             start=False, stop=True)
                            for h in range(HPC):
                                nc.vector.tensor_scalar_mul(
                                    v_aug[:, kt, h * 65:h * 65 + 64],
                                    ps[:, h * 64:(h + 1) * 64],
                                    v01_sb[:, kt:kt + 1])

                    def emit_qproj(b, q, half=None):
                        qcol = b * S + q * 512
                        w = 512 if half is None else 256
                        if half:
                            qcol += 256
                        xq_c = pxq.tile([128, KT, 512], bf16, tag="xq")
                        xq_c = xq_c[:, :, 0:w]
                        if rep > 0 and b == 0 and q == 0 and not half:
                            # rep chain: depend on previous rep's output
                            # (bit-garbage values; timing-only builds)
                            nc.sync.dma_start(
                                xq_c,
                                outT[:].bitcast(bf16)[:, 0:w]
                                .rearrange("(t p) r -> p t r", p=128))
                        else:
                            nc.sync.dma_start(
                                xq_c,
                                xqTf[:, qcol:qcol + w]
                                .rearrange("(t p) r -> p t r", p=128))
                        psf = psS.tile([128, 2 * 512], dt.float32, tag="s")
                        ps = psf[:, 0:w]
                        for t in range(KT):
                            nc.tensor.matmul(
                                ps, wqs_t[:, t], xq_c[:, t],
                                start=(t == 0), stop=(t == KT - 1))
                        nc.vector.tensor_scalar_add(
                            qT_h[:, qcol:qcol + w], ps, bqs_sb[:, 0:1])

                    CH = 3

                    def emit_att(b, q, filler=()):
                        # software pipeline: scores/exp of chunk c and
                        # attn@V of chunk c-1 share each phase, so the PE's
                        # attn@V work hides under the exp stream
                        qcol = b * S + q * 512
                        po = [psO.tile([65, 512], dt.float32, tag="o",
                                       name=f"po_h{h}")
                              for h in range(HPC)]
                        NCH = (KTB + CH - 1) // CH
                        pcs = {}
                        for c in range(NCH + 1):
                            if c < NCH:
                                p_chunk = pp.tile([128, CH, 2 * 512],
                                                  bf16, tag="pch")
                                pcs[c] = p_chunk
                                for kk in range(c * CH,
                                                min(c * CH + CH, KTB)):
                                    kcol = b * SKC + kk * 128
                                    pss = psS.tile([128, 2 * 512],
                                                   dt.float32, tag="s")
                                    for h in range(HPC):
                                        nc.tensor.matmul(
                                            pss[:, h * 512:(h + 1) * 512],
                                            kT_h[h * 64:(h + 1) * 64,
                                                 kcol:kcol + 128],
                                            qT_h[h * 64:(h + 1) * 64,
                                                 qcol:qcol + 512],
                                            start=True, stop=True,
                                            tile_position=(h * 64, 0))
                                    nc.scalar.activation(
                                        p_chunk[:, kk - c * CH], pss[:],
                                        AF.Exp)
                            if c > 0:
                                for kk in range((c - 1) * CH,
                                                min((c - 1) * CH + CH,
                                                    KTB)):
                                    for h in range(HPC):
                                        nc.tensor.matmul(
                                            po[h][:],
                                            v_aug[:, b * KTB + kk,
                                                  h * 65:(h + 1) * 65],
                                            pcs[c - 1][:, kk - (c - 1) * CH,
                                                       h * 512:(h + 1)
                                                       * 512],
                                            start=(kk == 0),
                                            stop=(kk == KTB - 1))
                            # PE slack while exp paces: slip filler work in
                            if filler and c < len(filler):
                                for th in filler[c]:
                                    th()
                        for h in range(HPC):
                            nc.vector.tensor_copy(
                                o65[h][:, qcol:qcol + 512], po[h][:])

                    # ---- prologue: first Q + batch-0 K (scores b0q0 can
                    # start at ~8us), then batch-0 V ----
                    emit_qproj(0, 0)
                    for ch in range(4):
                        emit_kchunk(ch)
                    for h in range(HPC):
                        nc.gpsimd.dma_start(
                            v_aug[:].rearrange(
                                "p a (h f) -> p a h f",
                                h=HPC)[:, :, h, 64:65]
                            .rearrange("p a one -> p (a one)"),
                            validm[:, 0:B * KTB])
                    for kt0 in range(0, min(9, B * KTB), 3):
                        emit_vchunk(kt0)

                    # ---- attention blocks with filler work in the chunk
                    # slots (PE slack while the exp stream paces Act);
                    # post[i][slot] runs after chunk `slot`'s attnV group ----
                    if KTB == 8:
                        post = {
                            0: [[("q", 0, 1, 0)], [("q", 0, 1, 1)],
                                [("k", 4)], []],
                            1: [[("q", 0, 2, 0)], [("q", 0, 2, 1)],
                                [("k", 5)], []],
                            2: [[("q", 0, 3, 0)], [("q", 0, 3, 1)],
                                [("k", 6)], []],
                            3: [[("q", 1, 0, 0)], [("q", 1, 0, 1), ("v", 9)],
                                [("k", 7)], [("v", 12)]],
                            4: [[("v", 15)], [("q", 1, 1, 0)],
                                [("q", 1, 1, 1)], []],
                            5: [[("q", 1, 2, 0)], [("q", 1, 2, 1)], [], []],
                            6: [[("q", 1, 3, 0)], [("q", 1, 3, 1)], [], []],
                        }
                    else:
                        # generic masks: all K/V in the prologue, only Q
                        # projections as fillers
                        for ch in range(4, 8):
                            emit_kchunk(ch)
                        for kt0 in range(9, B * KTB, 3):
                            emit_vchunk(kt0)
                        post = {
                            i: [[("q", (i + 1) // QT, (i + 1) % QT, 0)],
                                [("q", (i + 1) // QT, (i + 1) % QT, 1)]]
                            for i in range(B * QT - 1)
                        }

                    def mk(item):
                        if item[0] == "q":
                            return lambda: emit_qproj(item[1], item[2],
                                                      item[3])
                        if item[0] == "k":
                            return lambda: emit_kchunk(item[1])
                        return lambda: emit_vchunk(item[1])

                    blocks = [(b, q) for b in range(B) for q in range(QT)]
                    for i, (b, q) in enumerate(blocks):
                        emit_att(b, q, [[mk(it) for it in slot]
                                        for slot in post.get(i, [])])
                        if i == 3:
                            # first-half a2 payload (dests 0-3) ships early
                            for h in range(HPC):
                                nc.gpsimd.dma_start(
                                    a2_in[0:4, h * 65:(h + 1) * 65]
                                    .rearrange("d p r -> p d r"),
                                    o65[h][:, 0:4 * 512]
                                    .rearrange("p (d r) -> p d r", d=4))

                    for h in range(HPC):
                        nc.gpsimd.dma_start(
                            a2_in[4:8, h * 65:(h + 1) * 65]
                            .rearrange("d p r -> p d r"),
                            o65[h][:, 4 * 512:].rearrange("p (d r) -> p d r",
                                                          d=4))
                    # wo load here: transfers while the collective runs
                    nc.gpsimd.dma_start(
                        wo_t[:], wo[:].rearrange("(t p) n -> p t n", p=128))
                    nc.scalar.dma_start(
                        bo_sb[:], bo[:].rearrange("(n p) -> p n", p=128))

                if no_collective:
                    nc.sync.dma_start(a2_out[:], a2_in[:])
                else:
                    nc.gpsimd.collective_compute(
                        "AllToAll", mybir.AluOpType.bypass, replica_groups=rg,
                        ins=[a2_in.opt()], outs=[a2_out.opt()])

                # ---- normalize + out projection (row-sharded) ----
                with (
                    tc.tile_pool(name="p3a", bufs=1) as p3a,
                    tc.tile_pool(name="p3y", bufs=1) as p3y,
                    tc.tile_pool(name="p3ps", bufs=1, space="PSUM") as p3ps,
                    tc.tile_pool(name="p3sc", bufs=3, space="PSUM") as p3sc,
                ):
                    aT_sb = p3a.tile([128, KT, RPC], bf16, tag="aT")
                    nc.sync.dma_start(
                        aT_sb[0:64],
                        a2_out[:, 0:64].rearrange("j p r -> p j r"))
                    nc.sync.dma_start(
                        aT_sb[64:128],
                        a2_out[:, 65:129].rearrange("j p r -> p j r"))
                    den_sb = p3a.tile([16, RPC], bf16, tag="den")
                    nc.sync.dma_start(den_sb[0:8],
                                      a2_out[:, 64].rearrange("j r -> j r"))
                    nc.sync.dma_start(den_sb[8:16],
                                      a2_out[:, 129].rearrange("j r -> j r"))
                    rec_sb = p3a.tile([16, RPC], f32r, tag="rec")
                    with nc.allow_low_precision(
                            reason="1/den at fp22 is plenty"):
                        nc.vector.reciprocal(rec_sb[:], den_sb[:])
                    aN_sb = p3a.tile([128, KT, RPC], bf16, tag="aN")
                    yT_all = p3y.tile([128, NT, RPC], dt.float32, tag="y")
                    # normalize tile t, then immediately accumulate it into
                    # half the n-tiles (4 PSUM banks); second pass re-reads
                    yp = {}
                    for half in range(2):
                        for n in range(4 * half, 4 * half + 4):
                            yp[n] = p3ps.tile([128, RPC], dt.float32,
                                              tag=f"ps{n % 4}",
                                              name=f"yp_{n}")
                        for t in range(KT):
                            if half == 0:
                                sc = p3sc.tile([128, RPC], dt.float32,
                                               tag="sc")
                                nc.tensor.matmul(sc[:], selm_sb[:, t],
                                                 rec_sb[:],
                                                 start=True, stop=True)
                                nc.vector.tensor_mul(aN_sb[:, t],
                                                     aT_sb[:, t], sc[:])
                            for n in range(4 * half, 4 * half + 4):
                                nc.tensor.matmul(
                                    yp[n][:],
                                    wo_t[:, t, n * 128:(n + 1) * 128],
                                    aN_sb[:, t],
                                    start=(t == 0), stop=(t == KT - 1))
                        for n in range(4 * half, 4 * half + 4):
                            nc.vector.tensor_scalar_add(
                                yT_all[:, n], yp[n][:], bo_sb[:, n:n + 1])
                            nc.sync.dma_start(
                                outT[:].rearrange("(n p) r -> p n r",
                                                  p=128)[:, n],
                                yT_all[:, n])
                pw3.release()

    nc.compile()
    return nc


def _prep(query, key, value, mask, Wq, bq, Wk, bk, Wv, bv, Wo, bo):
    b16 = mybir.dt.np(mybir.dt.bfloat16)
    f = lambda a: np.ascontiguousarray(np.asarray(a, dtype=np.float32))
    m = np.asarray(mask).reshape(B, S)        # True = masked out

    # compact unmasked keys per batch, pad to a whole number of 128-tiles
    sels = [np.flatnonzero(~m[b]) for b in range(B)]
    nmax = max(1, max(len(s) for s in sels))
    _set_skc(128 * ((nmax + 127) // 128))
    key_np = f(key)
    val_np = f(value)
    ck = np.zeros((NKC, D), np.float32)
    cv = np.zeros((NKC, D), np.float32)
    valid = np.zeros(NKC, np.float32)
    for b in range(B):
        sel = sels[b]
        ck[b * SKC:b * SKC + len(sel)] = key_np[b, sel]
        cv[b * SKC:b * SKC + len(sel)] = val_np[b, sel]
        valid[b * SKC:b * SKC + len(sel)] = 1.0

    validm = np.ascontiguousarray(valid.reshape(B * KTB, 128).T)
    xqT = np.ascontiguousarray(f(query).reshape(R, D).T.astype(b16))
    xkT = np.ascontiguousarray(ck.T.astype(b16))
    xvT = np.ascontiguousarray(cv.T.astype(b16))

    # selm[r, t*128+m] = 1 iff r == t + 8*(m//64)
    selm = np.zeros((16, KT * 128), np.float32)
    for t in range(KT):
        for mm in range(128):
            selm[t + 8 * (mm // 64), t * 128 + mm] = 1.0

    wq_s = f(Wq) / np.float32(np.sqrt(DK))
    bq_s = f(bq) / np.float32(np.sqrt(DK))
    wk_f, wv_f, bk_f, bv_f = f(Wk), f(Wv), f(bk), f(bv)

    shared = {
        "xqTf": xqT, "xkTf": xkT, "xvTf": xvT,
        "wo": f(Wo).astype(b16), "bo": f(bo),
        "validm": validm.astype(b16),
        "validf": validm.astype(np.float32),
        "selm": selm,
        "onesb": np.ones((1, 128), b16),
    }
    in_maps = []
    for c in range(N_CORES):
        cols = slice(c * 128, (c + 1) * 128)
        in_maps.append({
            "wqs": np.ascontiguousarray(wq_s[:, cols].astype(b16)),
            "wks": np.ascontiguousarray(wk_f[:, cols].astype(b16)),
            "wvs": np.ascontiguousarray(wv_f[:, cols].astype(b16)),
            "bqs": np.ascontiguousarray(bq_s[cols].reshape(128, 1)),
            "bks": np.ascontiguousarray(bk_f[cols].reshape(128, 1)),
            "bvs": np.ascontiguousarray(bv_f[cols].reshape(1, 128)
                                        .astype(b16)),
            **shared,
        })
    return in_maps


def kernel(query, key, value, mask, Wq, bq, Wk, bk, Wv, bv, Wo, bo):
    # _prep first: it sets SKC (key-tile count) from the mask, which the
    # build specializes on
    in_maps = _prep(query, key, value, mask, Wq, bq, Wk, bk, Wv, bv, Wo, bo)
    if SKC not in _CACHE:
        _CACHE[SKC] = _build()
    nc = _CACHE[SKC]
    res = run_bass_kernel_spmd(nc, in_maps, list(range(N_CORES)))
    out = np.empty((R, D), np.float32)
    for c in range(N_CORES):
        out[c * RPC:(c + 1) * RPC] = res.results[c]["outT"].T
    return out.reshape(B, S, D)


# revision 60
# speedup vs baseline: 1.0466x; 1.0466x over previous
"""Multi-head attention layer on 8 trn2 NeuronCores — v4.

Structure: everything except the final out-projection is head-sharded
(2 of 16 heads per core), so the Q/K/V projections write their outputs
directly into the attention layout with NO collectives: each core
projects ALL tokens against its own 128-column weight slice (identical
PE cycles to row-sharding — work/8 either way), reading replicated
full inputs from HBM, which overlaps with compute from t=0.  A single
AllToAll converts the attention output to row-sharding for out_fc; it
carries UNNORMALIZED numerators plus the softmax denominators (row 65
of each head block), and normalization happens after the exchange in
the row-sharded domain (one reciprocal + per-tile broadcast matmuls),
keeping the attention inner loop free of DVE chains.

Key compaction: masked keys contribute exactly 0 to the output
(numerator and denominator), so the host drops them before the K/V
projections, padding per batch to a whole number of 128-key tiles (1024 for the
reference mask, vs S=2048).  Pad keys carry valid=0 which zeroes their
V rows and denominator column, so any mask stays exact; the build is
specialized per key-tile count and cached.

Phase-1 work is interleaved with batch-0 attention: batch-1 K/V
projections and later Q blocks are emitted between attention blocks so
their DMAs hide under compute; attention starts ~20us into the kernel.

Everything is bf16 except PSUM accumulation (fp32) and the fp32
output.  Scores are computed transposed (keys on partitions) so exp is
one ScalarE activation per key tile and attn@V needs no transposes.
"""
import numpy as np

from concourse import bacc, tile, mybir
from concourse.bass_utils import run_bass_kernel_spmd

N_CORES = 8
B, S, D, H = 2, 2048, 1024, 16
DK = D // H                      # 64
R = B * S                        # 4096 token rows
RPC = R // N_CORES               # 512 rows per core (phase-3 row-sharding)
HPC = H // N_CORES               # 2 heads per core
KT = D // 128                    # 8 contraction tiles
NT = D // 128                    # 8 output-dim tiles (out-projection)
QT = S // 512                    # 4 query blocks of 512 per batch

# compacted+padded keys per batch; set from the runtime mask by _prep()
# (ceil(max unmasked per batch / 128) tiles).  The build specializes on it.
SKC = 1024
KTB = SKC // 128                 # key tiles per batch
NKC = B * SKC                    # compacted keys total
KCH = NKC // 8                   # key-chunk width for the K projection


def _set_skc(skc):
    global SKC, KTB, NKC, KCH
    SKC = skc
    KTB = SKC // 128
    NKC = B * SKC
    KCH = NKC // 8

dt = mybir.dt
AF = mybir.ActivationFunctionType

_CACHE = {}


def _build(no_collective=False, reps=1):
    nc = bacc.Bacc("TRN2", target_bir_lowering=False, debug=False,
                   num_devices=N_CORES)

    # ---- kernel I/O (per-core tensors; x* are replicated full inputs,
    # w*/b* slices are this core's 128 head-dims) ----
    xqTf = nc.dram_tensor("xqTf", [D, R], dt.bfloat16, kind="ExternalInput")
    xkTf = nc.dram_tensor("xkTf", [D, NKC], dt.bfloat16, kind="ExternalInput")
    xvTf = nc.dram_tensor("xvTf", [D, NKC], dt.bfloat16, kind="ExternalInput")
    wqs = nc.dram_tensor("wqs", [D, 128], dt.bfloat16, kind="ExternalInput")
    wks = nc.dram_tensor("wks", [D, 128], dt.bfloat16, kind="ExternalInput")
    wvs = nc.dram_tensor("wvs", [D, 128], dt.bfloat16, kind="ExternalInput")
    wo = nc.dram_tensor("wo", [D, D], dt.bfloat16, kind="ExternalInput")
    bqs = nc.dram_tensor("bqs", [128, 1], dt.float32, kind="ExternalInput")
    bks = nc.dram_tensor("bks", [128, 1], dt.float32, kind="ExternalInput")
    bvs = nc.dram_tensor("bvs", [1, 128], dt.bfloat16, kind="ExternalInput")
    bo = nc.dram_tensor("bo", [D], dt.float32, kind="ExternalInput")
    # valid flags of compacted key kt*128+p: bf16 copy feeds v_aug's
    # denominator column, f32 copy feeds the V-row zeroing multiply
    validm = nc.dram_tensor("validm", [128, B * KTB], dt.bfloat16,
                            kind="ExternalInput")
    validf = nc.dram_tensor("validf", [128, B * KTB], dt.float32,
                            kind="ExternalInput")
    # selm[r, t, m] = 1 iff r == t + 8*(m//64): selects this tile's two
    # denominator rows for the phase-3 broadcast matmul
    selm = nc.dram_tensor("selm", [16, KT * 128], dt.float32,
                          kind="ExternalInput")
    onesb = nc.dram_tensor("onesb", [1, 128], dt.bfloat16,
                           kind="ExternalInput")
    outT = nc.dram_tensor("outT", [D, RPC], dt.float32, kind="ExternalOutput")

    f32r = dt.float32r
    bf16 = dt.bfloat16
    rg = [list(range(N_CORES))]

    with tile.TileContext(nc) as tc:
        with tc.tile_pool(name="dram", bufs=1, space="DRAM") as dram:
            for rep in range(reps):
                a2_in = dram.tile([N_CORES, 130, RPC], bf16)
                a2_out = dram.tile([N_CORES, 130, RPC], bf16)

                # out-projection weights (outlive the attention pools;
                # DMAs issued mid-attention so input loads go first)
                pw3 = tc.alloc_tile_pool(name="pw3", bufs=1)
                wo_t = pw3.tile([128, KT, D], bf16, tag="wo")
                bo_sb = pw3.tile([128, NT], dt.float32, tag="bo")
                selm_sb = pw3.tile([16, KT, 128], f32r, tag="selm")
                nc.scalar.dma_start(
                    selm_sb[:],
                    selm[:].rearrange("e (t m) -> e t m",
                                      t=KT).bitcast(f32r))

                with (
                    tc.tile_pool(name="pw", bufs=1) as pw,
                    tc.tile_pool(name="pxk", bufs=3) as pxk,
                    tc.tile_pool(name="pxv", bufs=3) as pxv,
                    tc.tile_pool(name="pxq", bufs=3) as pxq,
                    tc.tile_pool(name="pkv", bufs=1) as pkv,
                    tc.tile_pool(name="pp", bufs=5) as pp,
                    tc.tile_pool(name="psS", bufs=2, space="PSUM") as psS,
                    tc.tile_pool(name="psO", bufs=3, space="PSUM") as psO,
                    tc.tile_pool(name="psP", bufs=1, space="PSUM") as psP,
                ):
                    # ---- weights / biases / flags ----
                    wks_t = pw.tile([128, KT, 128], bf16, tag="wks")
                    wvs_t = pw.tile([128, KT, 128], bf16, tag="wvs")
                    wqs_t = pw.tile([128, KT, 128], bf16, tag="wqs")
                    nc.scalar.dma_start(
                        wks_t[:], wks[:].rearrange("(t p) n -> p t n", p=128))
                    nc.scalar.dma_start(
                        wvs_t[:], wvs[:].rearrange("(t p) n -> p t n", p=128))
                    nc.scalar.dma_start(
                        wqs_t[:], wqs[:].rearrange("(t p) n -> p t n", p=128))
                    bks_sb = pw.tile([128, 1], dt.float32, tag="bks")
                    bqs_sb = pw.tile([128, 1], dt.float32, tag="bqs")
                    bvs_sb = pw.tile([1, 128], bf16, tag="bvs")
                    nc.scalar.dma_start(bks_sb[:], bks[:])
                    nc.scalar.dma_start(bqs_sb[:], bqs[:])
                    nc.scalar.dma_start(bvs_sb[:], bvs[:])
                    onesb_sb = pw.tile([1, 128], bf16, tag="onesb")
                    nc.scalar.dma_start(onesb_sb[:], onesb[:])
                    v01_sb = pw.tile([128, B * KTB], dt.float32, tag="v01")
                    nc.scalar.dma_start(v01_sb[:], validf[:])

                    # ---- attention-layout outputs of the projections ----
                    kT_h = pkv.tile([128, NKC], bf16, tag="kh")
                    qT_h = pkv.tile([128, R], bf16, tag="qh")
                    # v_aug per key tile: [128, 130]: cols h*65+0..63 = V head
                    # h, col h*65+64 = valid flag (softmax denominator)
                    v_aug = pkv.tile([128, B * KTB, 130], bf16, tag="vh")
                    o65 = [pkv.tile([65, R], bf16, tag=f"o65_{h}",
                                    name=f"o65_{h}")
                           for h in range(HPC)]

                    def emit_kchunk(ch):
                        xk_c = pxk.tile([128, KT, KCH], bf16, tag="xk")
                        nc.sync.dma_start(
                            xk_c[:],
                            xkTf[:, ch * KCH:(ch + 1) * KCH]
                            .rearrange("(t p) r -> p t r", p=128))
                        psf = psP.tile([128, 512], dt.float32, tag="p")
                        ps = psf[:, 0:KCH]
                        for t in range(KT):
                            nc.tensor.matmul(
                                ps, wks_t[:, t], xk_c[:, t],
                                start=(t == 0), stop=(t == KT - 1))
                        nc.vector.tensor_scalar_add(
                            kT_h[:, ch * KCH:(ch + 1) * KCH], ps,
                            bks_sb[:, 0:1])

                    def emit_vchunk(kt0, nkt=3):
                        # one batched DMA, then per-tile projections
                        nkt = min(nkt, B * KTB - kt0)
                        xv_c = pxv.tile([128, KT, 3 * 128], bf16, tag="xv")
                        nc.gpsimd.dma_start(
                            xv_c[:, :, 0:nkt * 128],
                            xvTf[:, kt0 * 128:(kt0 + nkt) * 128]
                            .rearrange("(t p) r -> p t r", p=128))
                        for kk in range(nkt):
                            kt = kt0 + kk
                            psf = psP.tile([128, 512], dt.float32, tag="p")
                            ps = psf[:, 0:128]
                            for t in range(KT):
                                nc.tensor.matmul(
                                    ps, xv_c[:, t, kk * 128:(kk + 1) * 128],
                                    wvs_t[:, t],
                                    start=(t == 0), stop=False)
                            nc.tensor.matmul(ps, onesb_sb[:], bvs_sb[:],
                                             start=False, stop=True)
                            for h in range(HPC):
                                nc.vector.tensor_scalar_mul(
                                    v_aug[:, kt, h * 65:h * 65 + 64],
                                    ps[:, h * 64:(h + 1) * 64],
                                    v01_sb[:, kt:kt + 1])

                    def emit_qproj(b, q, half=None):
                        qcol = b * S + q * 512
                        w = 512 if half is None else 256
                        if half:
                            qcol += 256
                        xq_c = pxq.tile([128, KT, 512], bf16, tag="xq")
                        xq_c = xq_c[:, :, 0:w]
                        if rep > 0 and b == 0 and q == 0 and not half:
                            # rep chain: depend on previous rep's output
                            # (bit-garbage values; timing-only builds)
                            nc.sync.dma_start(
                                xq_c,
                                outT[:].bitcast(bf16)[:, 0:w]
                                .rearrange("(t p) r -> p t r", p=128))
                        else:
                            nc.sync.dma_start(
                                xq_c,
                                xqTf[:, qcol:qcol + w]
                                .rearrange("(t p) r -> p t r", p=128))
                        psf = psS.tile([128, 2 * 512], dt.float32, tag="s")
                        ps = psf[:, 0:w]
                        for t in range(KT):
                            nc.tensor.matmul(
                                ps, wqs_t[:, t], xq_c[:, t],
                                start=(t == 0), stop=(t == KT - 1))
                        nc.vector.tensor_scalar_add(
                            qT_h[:, qcol:qcol + w], ps, bqs_sb[:, 0:1])

                    CH = 3

                    def emit_att(b, q, filler=()):
                        # software pipeline: scores/exp of chunk c and
                        # attn@V of chunk c-1 share each phase, so the PE's
                        # attn@V work hides under the exp stream
                        qcol = b * S + q * 512
                        po = [psO.tile([65, 512], dt.float32, tag="o",
                                       name=f"po_h{h}")
                              for h in range(HPC)]
                        NCH = (KTB + CH - 1) // CH
                        pcs = {}
                        for c in range(NCH + 1):
                            if c < NCH:
                                p_chunk = pp.tile([128, CH, 2 * 512],
                                                  bf16, tag="pch")
                                pcs[c] = p_chunk
                                for kk in range(c * CH,
                                                min(c * CH + CH, KTB)):
                                    kcol = b * SKC + kk * 128
                                    pss = psS.tile([128, 2 * 512],
                                                   dt.float32, tag="s")
                                    for h in range(HPC):
                                        nc.tensor.matmul(
                                            pss[:, h * 512:(h + 1) * 512],
                                            kT_h[h * 64:(h + 1) * 64,
                                                 kcol:kcol + 128],
                                            qT_h[h * 64:(h + 1) * 64,
                                                 qcol:qcol + 512],
                                            start=True, stop=True,
                                            tile_position=(h * 64, 0))
                                    nc.scalar.activation(
                                        p_chunk[:, kk - c * CH], pss[:],
                                        AF.Exp)
                            if c > 0:
                                for kk in range((c - 1) * CH,
                                                min((c - 1) * CH + CH,
                                                    KTB)):
                                    for h in range(HPC):
                                        nc.tensor.matmul(
                                            po[h][:],
                                            v_aug[:, b * KTB + kk,
                                                  h * 65:(h + 1) * 65],
                                            pcs[c - 1][:, kk - (c - 1) * CH,
                                                       h * 512:(h + 1)
                                                       * 512],
                                            start=(kk == 0),
                                            stop=(kk == KTB - 1))
                            # PE slack while exp paces: slip filler work in
                            if filler and c < len(filler):
                                for th in filler[c]:
                                    th()
                        for h in range(HPC):
                            nc.vector.tensor_copy(
                                o65[h][:, qcol:qcol + 512], po[h][:])

                    # ---- prologue: first Q + batch-0 K (scores b0q0 can
                    # start at ~8us), then batch-0 V ----
                    emit_qproj(0, 0)
                    for h in range(HPC):
                        nc.gpsimd.dma_start(
                            v_aug[:].rearrange(
                                "p a (h f) -> p a h f",
                                h=HPC)[:, :, h, 64:65]
                            .rearrange("p a one -> p (a one)"),
                            validm[:, 0:B * KTB])
                    for ch in range(3):
                        emit_kchunk(ch)
                    emit_vchunk(0)
                    emit_kchunk(3)
                    for kt0 in range(3, min(9, B * KTB), 3):
                        emit_vchunk(kt0)

                    # ---- attention blocks with filler work in the chunk
                    # slots (PE slack while the exp stream paces Act);
                    # post[i][slot] runs after chunk `slot`'s attnV group ----
                    if KTB == 8:
                        post = {
                            0: [[("q", 0, 1, 0)], [("q", 0, 1, 1)],
                                [("k", 4)], []],
                            1: [[("q", 0, 2, 0)], [("q", 0, 2, 1)],
                                [("k", 5)], []],
                            2: [[("q", 0, 3, 0)], [("q", 0, 3, 1)],
                                [("k", 6)], []],
                            3: [[("q", 1, 0, 0)], [("q", 1, 0, 1), ("v", 9)],
                                [("k", 7)], [("v", 12)]],
                            4: [[("v", 15)], [("q", 1, 1, 0)],
                                [("q", 1, 1, 1)], []],
                            5: [[("q", 1, 2, 0)], [("q", 1, 2, 1)], [], []],
                            6: [[("q", 1, 3, 0)], [("q", 1, 3, 1)], [], []],
                        }
                    else:
                        # generic masks: all K/V in the prologue, only Q
                        # projections as fillers
                        for ch in range(4, 8):
                            emit_kchunk(ch)
                        for kt0 in range(9, B * KTB, 3):
                            emit_vchunk(kt0)
                        post = {
                            i: [[("q", (i + 1) // QT, (i + 1) % QT, 0)],
                                [("q", (i + 1) // QT, (i + 1) % QT, 1)]]
                            for i in range(B * QT - 1)
                        }

                    def mk(item):
                        if item[0] == "q":
                            return lambda: emit_qproj(item[1], item[2],
                                                      item[3])
                        if item[0] == "k":
                            return lambda: emit_kchunk(item[1])
                        return lambda: emit_vchunk(item[1])

                    blocks = [(b, q) for b in range(B) for q in range(QT)]
                    for i, (b, q) in enumerate(blocks):
                        emit_att(b, q, [[mk(it) for it in slot]
                                        for slot in post.get(i, [])])
                        if i == 3:
                            # first-half a2 payload (dests 0-3) ships early
                            for h in range(HPC):
                                nc.gpsimd.dma_start(
                                    a2_in[0:4, h * 65:(h + 1) * 65]
                                    .rearrange("d p r -> p d r"),
                                    o65[h][:, 0:4 * 512]
                                    .rearrange("p (d r) -> p d r", d=4))
                        elif i >= 4 and i < 7:
                            # each batch-1 dest ships as its block completes
                            for h in range(HPC):
                                nc.gpsimd.dma_start(
                                    a2_in[i:i + 1, h * 65:(h + 1) * 65]
                                    .rearrange("d p r -> p d r"),
                                    o65[h][:, i * 512:(i + 1) * 512]
                                    .rearrange("p (d r) -> p d r", d=1))

                    for h in range(HPC):
                        nc.gpsimd.dma_start(
                            a2_in[7:8, h * 65:(h + 1) * 65]
                            .rearrange("d p r -> p d r"),
                            o65[h][:, 7 * 512:].rearrange("p (d r) -> p d r",
                                                          d=1))
                    # wo load here: transfers while the collective runs
                    nc.gpsimd.dma_start(
                        wo_t[:], wo[:].rearrange("(t p) n -> p t n", p=128))
                    nc.scalar.dma_start(
                        bo_sb[:], bo[:].rearrange("(n p) -> p n", p=128))

                if no_collective:
                    nc.sync.dma_start(a2_out[:], a2_in[:])
                else:
                    nc.gpsimd.collective_compute(
                        "AllToAll", mybir.AluOpType.bypass, replica_groups=rg,
                        ins=[a2_in.opt()], outs=[a2_out.opt()])

                # ---- normalize + out projection (row-sharded) ----
                with (
                    tc.tile_pool(name="p3a", bufs=1) as p3a,
                    tc.tile_pool(name="p3y", bufs=1) as p3y,
                    tc.tile_pool(name="p3ps", bufs=1, space="PSUM") as p3ps,
                    tc.tile_pool(name="p3sc", bufs=3, space="PSUM") as p3sc,
                ):
                    aT_sb = p3a.tile([128, KT, RPC], bf16, tag="aT")
                    for j0 in (0, 4):
                        nc.sync.dma_start(
                            aT_sb[0:64, j0:j0 + 4],
                            a2_out[j0:j0 + 4, 0:64]
                            .rearrange("j p r -> p j r"))
                        nc.sync.dma_start(
                            aT_sb[64:128, j0:j0 + 4],
                            a2_out[j0:j0 + 4, 65:129]
                            .rearrange("j p r -> p j r"))
                    den_sb = p3a.tile([16, RPC], bf16, tag="den")
                    nc.sync.dma_start(den_sb[0:8],
                                      a2_out[:, 64].rearrange("j r -> j r"))
                    nc.sync.dma_start(den_sb[8:16],
                                      a2_out[:, 129].rearrange("j r -> j r"))
                    rec_sb = p3a.tile([16, RPC], f32r, tag="rec")
                    with nc.allow_low_precision(
                            reason="1/den at fp22 is plenty"):
                        nc.vector.reciprocal(rec_sb[:], den_sb[:])
                    aN_sb = p3a.tile([128, KT, RPC], bf16, tag="aN")
                    yT_all = p3y.tile([128, NT, RPC], dt.float32, tag="y")
                    # normalize tile t, then immediately accumulate it into
                    # half the n-tiles (4 PSUM banks); second pass re-reads
                    yp = {}
                    for half in range(2):
                        for n in range(4 * half, 4 * half + 4):
                            yp[n] = p3ps.tile([128, RPC], dt.float32,
                                              tag=f"ps{n % 4}",
                                              name=f"yp_{n}")
                        for t in range(KT):
                            if half == 0:
                                sc = p3sc.tile([128, RPC], dt.float32,
                                               tag="sc")
                                nc.tensor.matmul(sc[:], selm_sb[:, t],
                                                 rec_sb[:],
                                                 start=True, stop=True)
                                nc.vector.tensor_mul(aN_sb[:, t],
                                                     aT_sb[:, t], sc[:])
                            for n in range(4 * half, 4 * half + 4):
                                nc.tensor.matmul(
                                    yp[n][:],
                                    wo_t[:, t, n * 128:(n + 1) * 128],
                                    aN_sb[:, t],
                                    start=(t == 0), stop=(t == KT - 1))
                        for n in range(4 * half, 4 * half + 4):
                            nc.vector.tensor_scalar_add(
                                yT_all[:, n], yp[n][:], bo_sb[:, n:n + 1])
                            nc.sync.dma_start(
                                outT[:].rearrange("(n p) r -> p n r",
                                                  p=128)[:, n],
                                yT_all[:, n])
                pw3.release()

    nc.compile()
    return nc


def _prep(query, key, value, mask, Wq, bq, Wk, bk, Wv, bv, Wo, bo):
    b16 = mybir.dt.np(mybir.dt.bfloat16)
    f = lambda a: np.ascontiguousarray(np.asarray(a, dtype=np.float32))
    m = np.asarray(mask).reshape(B, S)        # True = masked out

    # compact unmasked keys per batch, pad to a whole number of 128-tiles
    sels = [np.flatnonzero(~m[b]) for b in range(B)]
    nmax = max(1, max(len(s) for s in sels))
    _set_skc(128 * ((nmax + 127) // 128))
    key_np = f(key)
    val_np = f(value)
    ck = np.zeros((NKC, D), np.float32)
    cv = np.zeros((NKC, D), np.float32)
    valid = np.zeros(NKC, np.float32)
    for b in range(B):
        sel = sels[b]
        ck[b * SKC:b * SKC + len(sel)] = key_np[b, sel]
        cv[b * SKC:b * SKC + len(sel)] = val_np[b, sel]
        valid[b * SKC:b * SKC + len(sel)] = 1.0

    validm = np.ascontiguousarray(valid.reshape(B * KTB, 128).T)
    xqT = np.ascontiguousarray(f(query).reshape(R, D).T.astype(b16))
    xkT = np.ascontiguousarray(ck.T.astype(b16))
    xvT = np.ascontiguousarray(cv.T.astype(b16))

    # selm[r, t*128+m] = 1 iff r == t + 8*(m//64)
    selm = np.zeros((16, KT * 128), np.float32)
    for t in range(KT):
        for mm in range(128):
            selm[t + 8 * (mm // 64), t * 128 + mm] = 1.0

    wq_s = f(Wq) / np.float32(np.sqrt(DK))
    bq_s = f(bq) / np.float32(np.sqrt(DK))
    wk_f, wv_f, bk_f, bv_f = f(Wk), f(Wv), f(bk), f(bv)

    shared = {
        "xqTf": xqT, "xkTf": xkT, "xvTf": xvT,
        "wo": f(Wo).astype(b16), "bo": f(bo),
        "validm": validm.astype(b16),
        "validf": validm.astype(np.float32),
        "selm": selm,
        "onesb": np.ones((1, 128), b16),
    }
    in_maps = []
    for c in range(N_CORES):
        cols = slice(c * 128, (c + 1) * 128)
        in_maps.append({
            "wqs": np.ascontiguousarray(wq_s[:, cols].astype(b16)),
            "wks": np.ascontiguousarray(wk_f[:, cols].astype(b16)),
            "wvs": np.ascontiguousarray(wv_f[:, cols].astype(b16)),
            "bqs": np.ascontiguousarray(bq_s[cols].reshape(128, 1)),
            "bks": np.ascontiguousarray(bk_f[cols].reshape(128, 1)),
            "bvs": np.ascontiguousarray(bv_f[cols].reshape(1, 128)
                                        .astype(b16)),
            **shared,
        })
    return in_maps


def kernel(query, key, value, mask, Wq, bq, Wk, bk, Wv, bv, Wo, bo):
    # _prep first: it sets SKC (key-tile count) from the mask, which the
    # build specializes on
    in_maps = _prep(query, key, value, mask, Wq, bq, Wk, bk, Wv, bv, Wo, bo)
    if SKC not in _CACHE:
        _CACHE[SKC] = _build()
    nc = _CACHE[SKC]
    res = run_bass_kernel_spmd(nc, in_maps, list(range(N_CORES)))
    out = np.empty((R, D), np.float32)
    for c in range(N_CORES):
        out[c * RPC:(c + 1) * RPC] = res.results[c]["outT"].T
    return out.reshape(B, S, D)


# revision 63
# speedup vs baseline: 4.4854x; 4.2859x over previous
"""Multi-head attention layer on 8 trn2 NeuronCores — v4.

Structure: everything except the final out-projection is head-sharded
(2 of 16 heads per core), so the Q/K/V projections write their outputs
directly into the attention layout with NO collectives: each core
projects ALL tokens against its own 128-column weight slice (identical
PE cycles to row-sharding — work/8 either way), reading replicated
full inputs from HBM, which overlaps with compute from t=0.  A single
AllToAll converts the attention output to row-sharding for out_fc; it
carries UNNORMALIZED numerators plus the softmax denominators (row 65
of each head block), and normalization happens after the exchange in
the row-sharded domain (one reciprocal + per-tile broadcast matmuls),
keeping the attention inner loop free of DVE chains.

Key compaction: masked keys contribute exactly 0 to the output
(numerator and denominator), so the host drops them before the K/V
projections, padding per batch to a whole number of 128-key tiles (1024 for the
reference mask, vs S=2048).  Pad keys carry valid=0 which zeroes their
V rows and denominator column, so any mask stays exact; the build is
specialized per key-tile count and cached.

Phase-1 work is interleaved with batch-0 attention: batch-1 K/V
projections and later Q blocks are emitted between attention blocks so
their DMAs hide under compute; attention starts ~20us into the kernel.

Everything is bf16 except PSUM accumulation (fp32) and the fp32
output.  Scores are computed transposed (keys on partitions) so exp is
one ScalarE activation per key tile and attn@V needs no transposes.
"""
import numpy as np

from concourse import bacc, tile, mybir
from concourse.bass_utils import run_bass_kernel_spmd

N_CORES = 8
B, S, D, H = 2, 2048, 1024, 16
DK = D // H                      # 64
R = B * S                        # 4096 token rows
RPC = R // N_CORES               # 512 rows per core (phase-3 row-sharding)
HPC = H // N_CORES               # 2 heads per core
KT = D // 128                    # 8 contraction tiles
NT = D // 128                    # 8 output-dim tiles (out-projection)
QT = S // 512                    # 4 query blocks of 512 per batch

# compacted+padded keys per batch; set from the runtime mask by _prep()
# (ceil(max unmasked per batch / 128) tiles).  The build specializes on it.
SKC = 1024
KTB = SKC // 128                 # key tiles per batch
NKC = B * SKC                    # compacted keys total
KCH = NKC // 8                   # key-chunk width for the K projection


def _set_skc(skc):
    global SKC, KTB, NKC, KCH
    SKC = skc
    KTB = SKC // 128
    NKC = B * SKC
    KCH = NKC // 8

dt = mybir.dt
AF = mybir.ActivationFunctionType

_CACHE = {}


def _build(no_collective=False, reps=1):
    nc = bacc.Bacc("TRN2", target_bir_lowering=False, debug=False,
                   num_devices=N_CORES)

    # ---- kernel I/O (per-core tensors; x* are replicated full inputs,
    # w*/b* slices are this core's 128 head-dims) ----
    xqTf = nc.dram_tensor("xqTf", [D, R], dt.bfloat16, kind="ExternalInput")
    xkTf = nc.dram_tensor("xkTf", [D, NKC], dt.bfloat16, kind="ExternalInput")
    xvTf = nc.dram_tensor("xvTf", [D, NKC], dt.bfloat16, kind="ExternalInput")
    wqs = nc.dram_tensor("wqs", [D, 128], dt.bfloat16, kind="ExternalInput")
    wks = nc.dram_tensor("wks", [D, 128], dt.bfloat16, kind="ExternalInput")
    wvs = nc.dram_tensor("wvs", [D, 128], dt.bfloat16, kind="ExternalInput")
    wo = nc.dram_tensor("wo", [D, D], dt.bfloat16, kind="ExternalInput")
    bqs = nc.dram_tensor("bqs", [128, 1], dt.float32, kind="ExternalInput")
    bks = nc.dram_tensor("bks", [128, 1], dt.float32, kind="ExternalInput")
    bvs = nc.dram_tensor("bvs", [1, 128], dt.bfloat16, kind="ExternalInput")
    bo = nc.dram_tensor("bo", [D], dt.float32, kind="ExternalInput")
    # valid flags of compacted key kt*128+p: bf16 copy feeds v_aug's
    # denominator column, f32 copy feeds the V-row zeroing multiply
    validm = nc.dram_tensor("validm", [128, B * KTB], dt.bfloat16,
                            kind="ExternalInput")
    validf = nc.dram_tensor("validf", [128, B * KTB], dt.float32,
                            kind="ExternalInput")
    # selm[r, t, m] = 1 iff r == t + 8*(m//64): selects this tile's two
    # denominator rows for the phase-3 broadcast matmul
    selm = nc.dram_tensor("selm", [16, KT * 128], dt.float32,
                          kind="ExternalInput")
    onesb = nc.dram_tensor("onesb", [1, 128], dt.bfloat16,
                           kind="ExternalInput")
    outT = nc.dram_tensor("outT", [D, RPC], dt.float32, kind="ExternalOutput")

    f32r = dt.float32r
    bf16 = dt.bfloat16
    rg = [list(range(N_CORES))]

    with tile.TileContext(nc) as tc:
        with tc.tile_pool(name="dram", bufs=1, space="DRAM") as dram:
            for rep in range(reps):
                a2_in = dram.tile([N_CORES, 130, RPC], bf16)
                a2_out = dram.tile([N_CORES, 130, RPC], bf16)

                # out-projection weights (outlive the attention pools;
                # DMAs issued mid-attention so input loads go first)
                pw3 = tc.alloc_tile_pool(name="pw3", bufs=1)
                wo_t = pw3.tile([128, KT, D], bf16, tag="wo")
                bo_sb = pw3.tile([128, NT], dt.float32, tag="bo")
                selm_sb = pw3.tile([16, KT, 128], f32r, tag="selm")
                nc.scalar.dma_start(
                    selm_sb[:],
                    selm[:].rearrange("e (t m) -> e t m",
                                      t=KT).bitcast(f32r))

                with (
                    tc.tile_pool(name="pw", bufs=1) as pw,
                    tc.tile_pool(name="pxk", bufs=3) as pxk,
                    tc.tile_pool(name="pxv", bufs=3) as pxv,
                    tc.tile_pool(name="pxq", bufs=3) as pxq,
                    tc.tile_pool(name="pkv", bufs=1) as pkv,
                    tc.tile_pool(name="pp", bufs=5) as pp,
                    tc.tile_pool(name="psS", bufs=2, space="PSUM") as psS,
                    tc.tile_pool(name="psO", bufs=3, space="PSUM") as psO,
                    tc.tile_pool(name="psP", bufs=1, space="PSUM") as psP,
                ):
                    # ---- weights / biases / flags ----
                    wks_t = pw.tile([128, KT, 128], bf16, tag="wks")
                    wvs_t = pw.tile([128, KT, 128], bf16, tag="wvs")
                    wqs_t = pw.tile([128, KT, 128], bf16, tag="wqs")
                    nc.scalar.dma_start(
                        wks_t[:], wks[:].rearrange("(t p) n -> p t n", p=128))
                    nc.scalar.dma_start(
                        wvs_t[:], wvs[:].rearrange("(t p) n -> p t n", p=128))
                    nc.scalar.dma_start(
                        wqs_t[:], wqs[:].rearrange("(t p) n -> p t n", p=128))
                    bks_sb = pw.tile([128, 1], dt.float32, tag="bks")
                    bqs_sb = pw.tile([128, 1], dt.float32, tag="bqs")
                    bvs_sb = pw.tile([1, 128], bf16, tag="bvs")
                    nc.scalar.dma_start(bks_sb[:], bks[:])
                    nc.scalar.dma_start(bqs_sb[:], bqs[:])
                    nc.scalar.dma_start(bvs_sb[:], bvs[:])
                    onesb_sb = pw.tile([1, 128], bf16, tag="onesb")
                    nc.scalar.dma_start(onesb_sb[:], onesb[:])
                    # warm the exp activation table before the first scores
                    dummy = pw.tile([1, 2], dt.float32, tag="dummy")
                    nc.scalar.activation(dummy[:], onesb_sb[0:1, 0:2],
                                         AF.Exp)
                    v01_sb = pw.tile([128, B * KTB], dt.float32, tag="v01")
                    nc.scalar.dma_start(v01_sb[:], validf[:])

                    # ---- attention-layout outputs of the projections ----
                    kT_h = pkv.tile([128, NKC], bf16, tag="kh")
                    qT_h = pkv.tile([128, R], bf16, tag="qh")
                    # v_aug per key tile: [128, 130]: cols h*65+0..63 = V head
                    # h, col h*65+64 = valid flag (softmax denominator)
                    v_aug = pkv.tile([128, B * KTB, 130], bf16, tag="vh")
                    o65 = [pkv.tile([65, R], bf16, tag=f"o65_{h}",
                                    name=f"o65_{h}")
                           for h in range(HPC)]

                    def emit_kchunk(ch):
                        xk_c = pxk.tile([128, KT, KCH], bf16, tag="xk")
                        nc.sync.dma_start(
                            xk_c[:],
                            xkTf[:, ch * KCH:(ch + 1) * KCH]
                            .rearrange("(t p) r -> p t r", p=128))
                        psf = psP.tile([128, 512], dt.float32, tag="p")
                        ps = psf[:, 0:KCH]
                        for t in range(KT):
                            nc.tensor.matmul(
                                ps, wks_t[:, t], xk_c[:, t],
                                start=(t == 0), stop=(t == KT - 1))
                        nc.vector.tensor_scalar_add(
                            kT_h[:, ch * KCH:(ch + 1) * KCH], ps,
                            bks_sb[:, 0:1])

                    def emit_vchunk(kt0, nkt=3):
                        # one batched DMA, then per-tile projections
                        nkt = min(nkt, B * KTB - kt0)
                        xv_c = pxv.tile([128, KT, 3 * 128], bf16, tag="xv")
                        nc.gpsimd.dma_start(
                            xv_c[:, :, 0:nkt * 128],
                            xvTf[:, kt0 * 128:(kt0 + nkt) * 128]
                            .rearrange("(t p) r -> p t r", p=128))
                        for kk in range(nkt):
                            kt = kt0 + kk
                            psf = psP.tile([128, 512], dt.float32, tag="p")
                            ps = psf[:, 0:128]
                            for t in range(KT):
                                nc.tensor.matmul(
                                    ps, xv_c[:, t, kk * 128:(kk + 1) * 128],
                                    wvs_t[:, t],
                                    start=(t == 0), stop=False)
                            nc.tensor.matmul(ps, onesb_sb[:], bvs_sb[:],
                                             start=False, stop=True)
                            for h in range(HPC):
                                nc.vector.tensor_scalar_mul(
                                    v_aug[:, kt, h * 65:h * 65 + 64],
                                    ps[:, h * 64:(h + 1) * 64],
                                    v01_sb[:, kt:kt + 1])

                    def emit_qproj(b, q, half=None):
                        qcol = b * S + q * 512
                        w = 512 if half is None else 256
                        if half:
                            qcol += 256
                        xq_c = pxq.tile([128, KT, 512], bf16, tag="xq")
                        xq_c = xq_c[:, :, 0:w]
                        if rep > 0 and b == 0 and q == 0 and not half:
                            # rep chain: depend on previous rep's output
                            # (bit-garbage values; timing-only builds)
                            nc.sync.dma_start(
                                xq_c,
                                outT[:].bitcast(bf16)[:, 0:w]
                                .rearrange("(t p) r -> p t r", p=128))
                        else:
                            nc.sync.dma_start(
                                xq_c,
                                xqTf[:, qcol:qcol + w]
                                .rearrange("(t p) r -> p t r", p=128))
                        psf = psS.tile([128, 2 * 512], dt.float32, tag="s")
                        ps = psf[:, 0:w]
                        for t in range(KT):
                            nc.tensor.matmul(
                                ps, wqs_t[:, t], xq_c[:, t],
                                start=(t == 0), stop=(t == KT - 1))
                        nc.vector.tensor_scalar_add(
                            qT_h[:, qcol:qcol + w], ps, bqs_sb[:, 0:1])

                    CH = 3

                    def emit_att(b, q, filler=()):
                        # software pipeline: scores/exp of chunk c and
                        # attn@V of chunk c-1 share each phase, so the PE's
                        # attn@V work hides under the exp stream
                        qcol = b * S + q * 512
                        po = [psO.tile([65, 512], dt.float32, tag="o",
                                       name=f"po_h{h}")
                              for h in range(HPC)]
                        NCH = (KTB + CH - 1) // CH
                        pcs = {}
                        for c in range(NCH + 1):
                            if c < NCH:
                                p_chunk = pp.tile([128, CH, 2 * 512],
                                                  bf16, tag="pch")
                                pcs[c] = p_chunk
                                for kk in range(c * CH,
                                                min(c * CH + CH, KTB)):
                                    kcol = b * SKC + kk * 128
                                    pss = psS.tile([128, 2 * 512],
                                                   dt.float32, tag="s")
                                    for h in range(HPC):
                                        nc.tensor.matmul(
                                            pss[:, h * 512:(h + 1) * 512],
                                            kT_h[h * 64:(h + 1) * 64,
                                                 kcol:kcol + 128],
                                            qT_h[h * 64:(h + 1) * 64,
                                                 qcol:qcol + 512],
                                            start=True, stop=True,
                                            tile_position=(h * 64, 0))
                                    nc.scalar.activation(
                                        p_chunk[:, kk - c * CH], pss[:],
                                        AF.Exp)
                            if c > 0:
                                for kk in range((c - 1) * CH,
                                                min((c - 1) * CH + CH,
                                                    KTB)):
                                    for h in range(HPC):
                                        nc.tensor.matmul(
                                            po[h][:],
                                            v_aug[:, b * KTB + kk,
                                                  h * 65:(h + 1) * 65],
                                            pcs[c - 1][:, kk - (c - 1) * CH,
                                                       h * 512:(h + 1)
                                                       * 512],
                                            start=(kk == 0),
                                            stop=(kk == KTB - 1))
                            # PE slack while exp paces: slip filler work in
                            if filler and c < len(filler):
                                for th in filler[c]:
                                    th()
                        for h in range(HPC):
                            nc.vector.tensor_copy(
                                o65[h][:, qcol:qcol + 512], po[h][:])

                    # ---- prologue: first Q + batch-0 K (scores b0q0 can
                    # start at ~8us), then batch-0 V ----
                    emit_qproj(0, 0)
                    for h in range(HPC):
                        nc.gpsimd.dma_start(
                            v_aug[:].rearrange(
                                "p a (h f) -> p a h f",
                                h=HPC)[:, :, h, 64:65]
                            .rearrange("p a one -> p (a one)"),
                            validm[:, 0:B * KTB])
                    for ch in range(3):
                        emit_kchunk(ch)
                    emit_vchunk(0)
                    emit_kchunk(3)
                    for kt0 in range(3, min(9, B * KTB), 3):
                        emit_vchunk(kt0)

                    # ---- attention blocks with filler work in the chunk
                    # slots (PE slack while the exp stream paces Act);
                    # post[i][slot] runs after chunk `slot`'s attnV group ----
                    if KTB == 8:
                        post = {
                            0: [[("q", 0, 1, 0)], [("q", 0, 1, 1)],
                                [("k", 4)], []],
                            1: [[("q", 0, 2, 0)], [("q", 0, 2, 1)],
                                [("k", 5)], []],
                            2: [[("q", 0, 3, 0)], [("q", 0, 3, 1)],
                                [("k", 6)], []],
                            3: [[("q", 1, 0, 0)], [("q", 1, 0, 1), ("v", 9)],
                                [("k", 7)], [("v", 12)]],
                            4: [[("v", 15)], [("q", 1, 1, 0)],
                                [("q", 1, 1, 1)], []],
                            5: [[("q", 1, 2, 0)], [("q", 1, 2, 1)], [], []],
                            6: [[("q", 1, 3, 0)], [("q", 1, 3, 1)], [], []],
                        }
                    else:
                        # generic masks: all K/V in the prologue, only Q
                        # projections as fillers
                        for ch in range(4, 8):
                            emit_kchunk(ch)
                        for kt0 in range(9, B * KTB, 3):
                            emit_vchunk(kt0)
                        post = {
                            i: [[("q", (i + 1) // QT, (i + 1) % QT, 0)],
                                [("q", (i + 1) // QT, (i + 1) % QT, 1)]]
                            for i in range(B * QT - 1)
                        }

                    def mk(item):
                        if item[0] == "q":
                            return lambda: emit_qproj(item[1], item[2],
                                                      item[3])
                        if item[0] == "k":
                            return lambda: emit_kchunk(item[1])
                        return lambda: emit_vchunk(item[1])

                    blocks = [(b, q) for b in range(B) for q in range(QT)]
                    for i, (b, q) in enumerate(blocks):
                        emit_att(b, q, [[mk(it) for it in slot]
                                        for slot in post.get(i, [])])
                        if i == 3:
                            # first-half a2 payload (dests 0-3) ships early
                            for h in range(HPC):
                                nc.gpsimd.dma_start(
                                    a2_in[0:4, h * 65:(h + 1) * 65]
                                    .rearrange("d p r -> p d r"),
                                    o65[h][:, 0:4 * 512]
                                    .rearrange("p (d r) -> p d r", d=4))
                        elif i >= 4 and i < 7:
                            # each batch-1 dest ships as its block completes
                            for h in range(HPC):
                                nc.gpsimd.dma_start(
                                    a2_in[i:i + 1, h * 65:(h + 1) * 65]
                                    .rearrange("d p r -> p d r"),
                                    o65[h][:, i * 512:(i + 1) * 512]
                                    .rearrange("p (d r) -> p d r", d=1))

                    for h in range(HPC):
                        nc.gpsimd.dma_start(
                            a2_in[7:8, h * 65:(h + 1) * 65]
                            .rearrange("d p r -> p d r"),
                            o65[h][:, 7 * 512:].rearrange("p (d r) -> p d r",
                                                          d=1))
                    # wo load here: transfers while the collective runs
                    nc.gpsimd.dma_start(
                        wo_t[:], wo[:].rearrange("(t p) n -> p t n", p=128))
                    nc.scalar.dma_start(
                        bo_sb[:], bo[:].rearrange("(n p) -> p n", p=128))

                if no_collective:
                    nc.sync.dma_start(a2_out[:], a2_in[:])
                else:
                    nc.gpsimd.collective_compute(
                        "AllToAll", mybir.AluOpType.bypass, replica_groups=rg,
                        ins=[a2_in.opt()], outs=[a2_out.opt()])

                # ---- normalize + out projection (row-sharded) ----
                with (
                    tc.tile_pool(name="p3a", bufs=1) as p3a,
                    tc.tile_pool(name="p3y", bufs=1) as p3y,
                    tc.tile_pool(name="p3ps", bufs=1, space="PSUM") as p3ps,
                    tc.tile_pool(name="p3sc", bufs=3, space="PSUM") as p3sc,
                ):
                    aT_sb = p3a.tile([128, KT, RPC], bf16, tag="aT")
                    for j0 in (0, 4):
                        nc.sync.dma_start(
                            aT_sb[0:64, j0:j0 + 4],
                            a2_out[j0:j0 + 4, 0:64]
                            .rearrange("j p r -> p j r"))
                        nc.sync.dma_start(
                            aT_sb[64:128, j0:j0 + 4],
                            a2_out[j0:j0 + 4, 65:129]
                            .rearrange("j p r -> p j r"))
                    den_sb = p3a.tile([16, RPC], bf16, tag="den")
                    nc.sync.dma_start(den_sb[0:8],
                                      a2_out[:, 64].rearrange("j r -> j r"))
                    nc.sync.dma_start(den_sb[8:16],
                                      a2_out[:, 129].rearrange("j r -> j r"))
                    rec_sb = p3a.tile([16, RPC], f32r, tag="rec")
                    with nc.allow_low_precision(
                            reason="1/den at fp22 is plenty"):
                        nc.vector.reciprocal(rec_sb[:], den_sb[:])
                    aN_sb = p3a.tile([128, KT, RPC], bf16, tag="aN")
                    yT_all = p3y.tile([128, NT, RPC], dt.float32, tag="y")
                    # normalize tile t, then immediately accumulate it into
                    # half the n-tiles (4 PSUM banks); second pass re-reads
                    yp = {}
                    for half in range(2):
                        for n in range(4 * half, 4 * half + 4):
                            yp[n] = p3ps.tile([128, RPC], dt.float32,
                                              tag=f"ps{n % 4}",
                                              name=f"yp_{n}")
                        for t in range(KT):
                            if half == 0:
                                sc = p3sc.tile([128, RPC], dt.float32,
                                               tag="sc")
                                nc.tensor.matmul(sc[:], selm_sb[:, t],
                                                 rec_sb[:],
                                                 start=True, stop=True)
                                nc.vector.tensor_mul(aN_sb[:, t],
                                                     aT_sb[:, t], sc[:])
                            for n in range(4 * half, 4 * half + 4):
                                nc.tensor.matmul(
                                    yp[n][:],
                                    wo_t[:, t, n * 128:(n + 1) * 128],
                                    aN_sb[:, t],
                                    start=(t == 0), stop=(t == KT - 1))
                        for n in range(4 * half, 4 * half + 4):
                            nc.vector.tensor_scalar_add(
                                yT_all[:, n], yp[n][:], bo_sb[:, n:n + 1])
                            nc.sync.dma_start(
                                outT[:].rearrange("(n p) r -> p n r",
                                                  p=128)[:, n],
                                yT_all[:, n])
                pw3.release()

    nc.compile()
    return nc


def _prep(query, key, value, mask, Wq, bq, Wk, bk, Wv, bv, Wo, bo):
    b16 = mybir.dt.np(mybir.dt.bfloat16)
    f = lambda a: np.ascontiguousarray(np.asarray(a, dtype=np.float32))
    m = np.asarray(mask).reshape(B, S)        # True = masked out

    # compact unmasked keys per batch, pad to a whole number of 128-tiles
    sels = [np.flatnonzero(~m[b]) for b in range(B)]
    nmax = max(1, max(len(s) for s in sels))
    _set_skc(128 * ((nmax + 127) // 128))
    key_np = f(key)
    val_np = f(value)
    ck = np.zeros((NKC, D), np.float32)
    cv = np.zeros((NKC, D), np.float32)
    valid = np.zeros(NKC, np.float32)
    for b in range(B):
        sel = sels[b]
        ck[b * SKC:b * SKC + len(sel)] = key_np[b, sel]
        cv[b * SKC:b * SKC + len(sel)] = val_np[b, sel]
        valid[b * SKC:b * SKC + len(sel)] = 1.0

    validm = np.ascontiguousarray(valid.reshape(B * KTB, 128).T)
    xqT = np.ascontiguousarray(f(query).reshape(R, D).T.astype(b16))
    xkT = np.ascontiguousarray(ck.T.astype(b16))
    xvT = np.ascontiguousarray(cv.T.astype(b16))

    # selm[r, t*128+m] = 1 iff r == t + 8*(m//64)
    selm = np.zeros((16, KT * 128), np.float32)
    for t in range(KT):
        for mm in range(128):
            selm[t + 8 * (mm // 64), t * 128 + mm] = 1.0

    wq_s = f(Wq) / np.float32(np.sqrt(DK))
    bq_s = f(bq) / np.float32(np.sqrt(DK))
    wk_f, wv_f, bk_f, bv_f = f(Wk), f(Wv), f(bk), f(bv)

    shared = {
        "xqTf": xqT, "xkTf": xkT, "xvTf": xvT,
        "wo": f(Wo).astype(b16), "bo": f(bo),
        "validm": validm.astype(b16),
        "validf": validm.astype(np.float32),
        "selm": selm,
        "onesb": np.ones((1, 128), b16),
    }
    in_maps = []
    for c in range(N_CORES):
        cols = slice(c * 128, (c + 1) * 128)
        in_maps.append({
            "wqs": np.ascontiguousarray(wq_s[:, cols].astype(b16)),
            "wks": np.ascontiguousarray(wk_f[:, cols].astype(b16)),
            "wvs": np.ascontiguousarray(wv_f[:, cols].astype(b16)),
            "bqs": np.ascontiguousarray(bq_s[cols].reshape(128, 1)),
            "bks": np.ascontiguousarray(bk_f[cols].reshape(128, 1)),
            "bvs": np.ascontiguousarray(bv_f[cols].reshape(1, 128)
                                        .astype(b16)),
            **shared,
        })
    return in_maps


def kernel(query, key, value, mask, Wq, bq, Wk, bk, Wv, bv, Wo, bo):
    # _prep first: it sets SKC (key-tile count) from the mask, which the
    # build specializes on
    in_maps = _prep(query, key, value, mask, Wq, bq, Wk, bk, Wv, bv, Wo, bo)
    if SKC not in _CACHE:
        _CACHE[SKC] = _build()
    nc = _CACHE[SKC]
    res = run_bass_kernel_spmd(nc, in_maps, list(range(N_CORES)))
    out = np.empty((R, D), np.float32)
    for c in range(N_CORES):
        out[c * RPC:(c + 1) * RPC] = res.results[c]["outT"].T
    return out.reshape(B, S, D)
